# revision 27
# baseline (speedup 1.0000x reference)
"""Trainium2 Bass kernel for nn_Decoder (embed -> LSTM -> vocab projection).

v3 layout (8 NeuronCores, single SPMD NEFF):
  - Host: embedding gather + concat -> lstm_in; pre-transpose weights and
    permute the 4096 gate columns into (half, sub, gate, j) order so the
    LSTM gate matmuls can be packed 4x into PE column groups.
  - gx GEMM rows-layout as v2: gx[320,4096] = x @ W_ih^T + b.
  - LSTM recurrence: per half (512 h-cols), ONE psum [128,512] holds 4
    column-group strips; strip cg = h-subchunk cg with free dim (gate,j).
    The 4 strips' matmuls run concurrently on the PE (col-group tiling),
    cutting per-step PE time ~4x vs v2. Activations/elemwise run at full
    128-partition width. One 128x128 PE transpose per half rebuilds h_T.
  - Per-timestep AllGather of h_T; FC m-tiles [128 rows] gather directly
    from the AG buffer (no staging tiles), fc_W shard resident in SBUF.
  - FC vocab-sharded: logits[2560,3750] = hs @ fc_W_shard^T + fc_b,
    interleaved into LSTM steps to keep the PE dense, bulk after step 9.
  - Host: undo row permutation, concat vocab shards.
"""
import ml_dtypes
import numpy as np
import jax
from jax.sharding import Mesh, PartitionSpec
from jax.experimental.shard_map import shard_map

import concourse.bass as bass
import concourse.mybir as mybir
import concourse.tile as tile
from concourse import bacc
from concourse.bass2jax import _bass_exec_p, install_neuronx_cc_hook, partition_id_tensor
from concourse.masks import make_identity

P = 128
NCORES = 8
B, T, FEAT, EMB, HID, VOCAB = 256, 10, 512, 512, 1024, 30000
DIN = FEAT + EMB          # 1024
G = 4 * HID               # 4096
BL = B // NCORES          # 32 batches per core
RL = BL * T               # 320 rows per core (t-major: r = t*BL + b)
RA = 256 * T              # 2560 rows total (t-major: r = t*256 + j*128 + a%4*32 + b)
VL = VOCAB // NCORES      # 3750 vocab per core
KT = DIN // P             # 8 contraction tiles
NVC = (VL + 511) // 512   # 8 vocab chunks (7x512 + 166)
F32 = mybir.dt.float32
BF16 = mybir.dt.bfloat16
Act = mybir.ActivationFunctionType

_CACHE = {}


def _build_nc():
    nc = bacc.Bacc("TRN2", target_bir_lowering=False, debug=False, num_devices=NCORES)
    x_a = nc.dram_tensor("x_a", [P, KT * RL], BF16, kind="ExternalInput").ap()
    w_ih_b = nc.dram_tensor("w_ih_b", [8, P, KT * 512], BF16,
                            kind="ExternalInput").ap()
    w_hh_a = nc.dram_tensor("w_hh_a", [P, KT * G], BF16, kind="ExternalInput").ap()
    bias_rep = nc.dram_tensor("bias_rep", [P, G], BF16, kind="ExternalInput").ap()
    fc_w_a = nc.dram_tensor("fc_w_a", [P, KT * VL], BF16, kind="ExternalInput").ap()
    fc_b_rep = nc.dram_tensor("fc_b_rep", [P, VL], BF16, kind="ExternalInput").ap()
    logits = nc.dram_tensor("logits", [RA, VL], F32, kind="ExternalOutput").ap()

    MT_X = [(0, 128), (128, 128), (256, 64)]  # (row0, rows) m-tiles of 320
    CH = 2048                                 # dma chunk: elements per partition

    with tile.TileContext(nc) as tc:
        with tc.tile_pool(name="dram", bufs=1, space="DRAM") as dram_pool:
            hs_dram = dram_pool.tile([T, HID, BL], BF16)
            ag_outs = [dram_pool.tile([NCORES, HID, BL], BF16,
                                      addr_space="Shared", name=f"ag_{t}")
                       for t in range(T)]
            gx_dram = dram_pool.tile([3, P, G], BF16)
            warm_in = dram_pool.tile([1, 64], BF16)
            warm_out = dram_pool.tile([NCORES, 64], BF16, addr_space="Shared",
                                      name="warm_ag")

            with tc.tile_pool(name="persist", bufs=1) as persist, \
                 tc.tile_pool(name="whh_pool", bufs=1) as whh_pool, \
                 tc.tile_pool(name="fcw_pool", bufs=1) as fcw_pool, \
                 tc.tile_pool(name="hsmt_pool", bufs=3) as hsmt_pool, \
                 tc.tile_pool(name="gates_pool", bufs=2) as gates_pool, \
                 tc.tile_pool(name="step_pool", bufs=2) as step_pool, \
                 tc.tile_pool(name="gxb_pool", bufs=2) as gxb_pool, \
                 tc.tile_pool(name="fc_out", bufs=3) as fc_out, \
                 tc.tile_pool(name="gch_psum", bufs=2, space="PSUM") as gch_psum, \
                 tc.tile_pool(name="tp_psum", bufs=2, space="PSUM") as tp_psum:
                ident_f = persist.tile([P, P], F32)
                make_identity(nc, ident_f[:])
                ident_b = persist.tile([P, P], BF16)
                nc.vector.tensor_copy(ident_b[:], ident_f[:])
                gx_t0 = persist.tile([32, G], BF16)
                h_Tb = persist.tile([P, KT, BL], BF16)
                c_sb = persist.tile([P, 2, P], F32)
                whh_sb = whh_pool.tile([P, KT, G], BF16)
                fcw_sb = fcw_pool.tile([P, KT, VL], BF16)
                fcb_sb = persist.tile([P, VL], BF16)

                whh_flat = whh_sb[:].rearrange("p k g -> p (k g)")
                fcw_flat = fcw_sb[:].rearrange("p k v -> p (k v)")

                # warmup collective: aligns core launch skew off the
                # critical path so AG_0 doesn't pay it after step 0
                nc.gpsimd.dma_start(warm_in[:], x_a[0:1, 0:64])
                nc.gpsimd.collective_compute(
                    "AllGather", mybir.AluOpType.bypass,
                    replica_groups=[list(range(NCORES))],
                    ins=[warm_in.opt()], outs=[warm_out.opt()])

                # ---- FC machinery (vocab-sharded, m-tile = (t, j)) ----
                def load_mtile(t, j):
                    hsmt = hsmt_pool.tile([P, KT, P], BF16,
                                          name=f"hsmt_{t}_{j}", tag="hsmt")
                    for a in range(4):
                        nc.sync.dma_start(
                            hsmt[:, :, 32 * a:32 * a + 32],
                            ag_outs[t][4 * j + a].rearrange("(k p) b -> p k b", p=P))
                    return hsmt

                in_drain = [False]

                def emit_group(hsmt, t, j, v):
                    w = min(512, VL - 512 * v)
                    ps = fc_pools["psum"].tile([P, 512], F32,
                                               name=f"fps_{t}_{j}_{v}", tag="fps")
                    for k in range(KT):
                        nc.tensor.matmul(
                            ps[:, 0:w], hsmt[:, k, :],
                            fcw_sb[:, k, 512 * v:512 * v + w],
                            start=(k == 0), stop=(k == KT - 1))
                    ot = fc_out.tile([P, 512], F32, name=f"fo_{t}_{j}_{v}", tag="fo")
                    nc.vector.tensor_add(ot[:, 0:w], ps[:, 0:w],
                                         fcb_sb[:, 512 * v:512 * v + w])
                    eng = nc.gpsimd if (in_drain[0] and v % 2 == 1) else nc.scalar
                    eng.dma_start(
                        logits[256 * t + P * j:256 * t + P * (j + 1),
                               512 * v:512 * v + w],
                        ot[:, 0:w])

                fc_pools = {}
                fc_state = {"emit": 0, "v": 0, "pref": 0}
                fc_tiles = {}

                def prefetch_fc(t_limit):
                    # issue gather DMAs up to 3 m-tiles ahead (pool WAR paces)
                    while (fc_state["pref"] < 2 * T
                           and fc_state["pref"] - fc_state["emit"] < 3
                           and fc_state["pref"] // 2 <= t_limit):
                        u = fc_state["pref"]
                        fc_tiles[u] = load_mtile(u // 2, u % 2)
                        fc_state["pref"] += 1

                def fc_lim(t):
                    return t - 5 if fc_state["emit"] == 0 else t - 4

                def emit_fc(n_groups, t_limit):
                    done = 0
                    while done < n_groups and fc_state["emit"] < 2 * T:
                        u = fc_state["emit"]
                        t, j = u // 2, u % 2
                        if t > t_limit or u not in fc_tiles:
                            return
                        emit_group(fc_tiles[u], t, j, fc_state["v"])
                        done += 1
                        fc_state["v"] += 1
                        if fc_state["v"] == NVC:
                            fc_state["v"] = 0
                            del fc_tiles[u]
                            fc_state["emit"] += 1

                # ---- Phase A: gx = x @ W_ih^T + b   (rows x gates, bf16) ----
                with tc.tile_pool(name="phA", bufs=1) as phA, \
                     tc.tile_pool(name="wih_pool", bufs=2) as wih_pool, \
                     tc.tile_pool(name="gx_stage", bufs=4) as gx_stage, \
                     tc.tile_pool(name="gx_psum", bufs=2, space="PSUM") as gx_psum:
                    x_sb = phA.tile([P, KT, RL], BF16)
                    x_flat = x_sb[:].rearrange("p k r -> p (k r)")
                    bias_sb = phA.tile([P, G], BF16)
                    # x first (gx starts on it), then wih/whh interleave on
                    # both rings, bias after the first wih pair, fcw + fcb
                    # trickle behind the gx loop.
                    for c0 in range(0, KT * RL, CH):
                        nc.scalar.dma_start(x_flat[:, c0:min(c0 + CH, KT * RL)],
                                            x_a[:, c0:min(c0 + CH, KT * RL)])

                    whh_ci = [0]

                    def trickle_whh(k):
                        for _ in range(k):
                            if whh_ci[0] >= KT * G // CH:
                                return
                            c0 = whh_ci[0] * CH
                            eng = nc.sync if whh_ci[0] % 2 == 0 else nc.scalar
                            eng.dma_start(whh_flat[:, c0:c0 + CH],
                                          w_hh_a[:, c0:c0 + CH])
                            whh_ci[0] += 1

                    def load_wih(n):
                        wt = wih_pool.tile([P, KT, 512], BF16,
                                           name=f"wih_{n}", tag="wih")
                        wf = wt[:].rearrange("p k v -> p (k v)")
                        # split each tile's chunks across BOTH rings so the
                        # tile's serial transfer time on one ring halves
                        for ci, c0 in enumerate(range(0, KT * 512, CH)):
                            eng = nc.sync if (n + ci) % 2 == 0 else nc.scalar
                            eng.dma_start(wf[:, c0:c0 + CH],
                                          w_ih_b[n][:, c0:c0 + CH])
                        return wt

                    nfc = (KT * VL + CH - 1) // CH
                    fcw_ci = [0]

                    def trickle_fcw(k):
                        # ring-paced: these sit behind WAR-gated wih loads /
                        # data-gated evictions, so they enqueue progressively
                        # instead of flooding the hardware queues upfront.
                        for _ in range(k):
                            if fcw_ci[0] >= nfc:
                                return
                            c0 = fcw_ci[0] * CH
                            c1 = min(c0 + CH, KT * VL)
                            eng = nc.sync if fcw_ci[0] % 2 == 0 else nc.scalar
                            eng.dma_start(fcw_flat[:, c0:c1], fc_w_a[:, c0:c1])
                            fcw_ci[0] += 1

                    for c0 in range(0, G, CH):
                        nc.scalar.dma_start(bias_sb[:, c0:c0 + CH],
                                            bias_rep[:, c0:c0 + CH])
                    wts = [load_wih(0), load_wih(1)]
                    trickle_whh(2)
                    for n in range(8):
                        wt = wts[n]
                        if n + 2 < 8:
                            wts.append(load_wih(n + 2))
                        trickle_whh(2)
                        for mi, (r0, rn) in enumerate(MT_X):
                            ps = gx_psum.tile([P, 512], F32,
                                              name=f"gxps_{n}_{mi}", tag="gxps")
                            for k in range(KT):
                                nc.tensor.matmul(
                                    ps[0:rn, :], x_sb[:, k, r0:r0 + rn],
                                    wt[:, k, :],
                                    start=(k == 0), stop=(k == KT - 1))
                            gt = gx_stage.tile([P, 512], BF16,
                                               name=f"gxs_{n}_{mi}", tag="gxs")
                            nc.vector.tensor_add(
                                gt[0:rn, :],
                                ps[0:rn, :], bias_sb[0:rn, n * 512:(n + 1) * 512])
                            if mi == 0:
                                nc.vector.tensor_copy(
                                    gx_t0[:, n * 512:(n + 1) * 512], gt[0:32, :])
                            nc.scalar.dma_start(
                                gx_dram[mi, 0:rn, n * 512:(n + 1) * 512], gt[0:rn, :])
                        trickle_fcw(2)
                        if n == 7:
                            nc.sync.dma_start(fcb_sb[:, 0:CH],
                                              fc_b_rep[:, 0:CH])
                            nc.scalar.dma_start(fcb_sb[:, CH:VL],
                                                fc_b_rep[:, CH:VL])
                    trickle_fcw(nfc)

                # ---- Phase B: LSTM recurrence, FC interleaved ----
                fc_psum_cm = tc.tile_pool(name="fc_psum", bufs=3, space="PSUM")
                fc_pools["psum"] = fc_psum_cm.__enter__()
                for t in range(T):
                    mt, jj = t // 4, t % 4
                    if t > 0:
                        gxt = gxb_pool.tile([32, G], BF16,
                                            name=f"gxt_{t}", tag="gxt")
                        nc.scalar.dma_start(
                            gxt[:], gx_dram[mt, 32 * jj:32 * jj + 32, :])
                        src = gxt
                    else:
                        src = gx_t0

                    gates = {}
                    for half in (0, 1):
                        ps = gch_psum.tile([P, 512], F32,
                                           name=f"gps_{t}_{half}", tag="gps")
                        for cg in range(4):
                            nc.tensor.matmul(
                                ps[32 * cg:32 * cg + 32, :],
                                ident_b[0:32, 0:32],
                                src[:, half * 2048 + cg * 512:
                                    half * 2048 + (cg + 1) * 512],
                                start=True, stop=(t == 0),
                                tile_position=(0, 32 * cg))
                        if t > 0:
                            for k in range(KT):
                                for cg in range(4):
                                    nc.tensor.matmul(
                                        ps[32 * cg:32 * cg + 32, :],
                                        h_Tb[:, k, :],
                                        whh_sb[:, k, half * 2048 + cg * 512:
                                               half * 2048 + (cg + 1) * 512],
                                        start=False, stop=(k == KT - 1),
                                        tile_position=(0, 32 * cg))
                        g_sb = gates_pool.tile([P, 512], F32,
                                               name=f"gates_{t}_{half}", tag="ga")
                        nc.scalar.activation(g_sb[:, 256:384], ps[:, 256:384],
                                             Act.Tanh)
                        nc.scalar.activation(g_sb[:, 0:256], ps[:, 0:256],
                                             Act.Sigmoid)
                        nc.scalar.activation(g_sb[:, 384:512], ps[:, 384:512],
                                             Act.Sigmoid)
                        gates[half] = g_sb

                    emit_fc(1, fc_lim(t))
                    for half in (0, 1):
                        g_sb = gates[half]
                        tmp = step_pool.tile([P, P], F32,
                                             name=f"tmp_{t}_{half}", tag="tmp")
                        nc.vector.tensor_mul(tmp[:], g_sb[:, 0:128],
                                             g_sb[:, 256:384])
                        if t == 0:
                            nc.vector.tensor_copy(c_sb[:, half, :], tmp[:])
                        else:
                            nc.vector.tensor_mul(c_sb[:, half, :],
                                                 g_sb[:, 128:256],
                                                 c_sb[:, half, :])
                            nc.vector.tensor_add(c_sb[:, half, :],
                                                 c_sb[:, half, :], tmp[:])
                        th = step_pool.tile([P, P], F32,
                                            name=f"th_{t}_{half}", tag="th")
                        nc.scalar.activation(th[:], c_sb[:, half, :], Act.Tanh)
                        h_hf = step_pool.tile([P, P], BF16,
                                              name=f"h_{t}_{half}", tag="h")
                        nc.vector.tensor_mul(h_hf[:], g_sb[:, 384:512], th[:])
                        tp = tp_psum.tile([P, P], BF16,
                                          name=f"tp_{t}_{half}", tag="tp")
                        nc.tensor.transpose(tp[:], h_hf[:], ident_b[:])
                        nc.vector.tensor_copy(
                            h_Tb[:, 4 * half:4 * half + 4, :],
                            tp[:].rearrange("p (k b) -> p k b", k=4))
                        nc.scalar.dma_start(
                            hs_dram[t].rearrange(
                                "(k p) b -> p k b", p=P)[:, 4 * half:4 * half + 4, :],
                            h_Tb[:, 4 * half:4 * half + 4, :])
                        if half == 0:
                            emit_fc(1, fc_lim(t))

                    nc.gpsimd.collective_compute(
                        "AllGather", mybir.AluOpType.bypass,
                        replica_groups=[list(range(NCORES))],
                        ins=[hs_dram[t].opt()], outs=[ag_outs[t].opt()])
                    prefetch_fc(t - 2)
                    emit_fc(2, fc_lim(t))

                # ---- Phase C: drain remaining FC work ----
                in_drain[0] = True
                while fc_state["emit"] < 2 * T:
                    prefetch_fc(T - 1)
                    emit_fc(1, T - 1)
                fc_psum_cm.__exit__(None, None, None)
    nc.compile()
    return nc


def _build_sharded(nc, n_cores=NCORES):
    install_neuronx_cc_hook()
    partition_name = nc.partition_id_tensor.name if nc.partition_id_tensor else None
    in_names, out_names, out_avals, zero_shapes = [], [], [], []
    for alloc in nc.m.functions[0].allocations:
        if not isinstance(alloc, mybir.MemoryLocationSet):
            continue
        name = alloc.memorylocations[0].name
        if alloc.kind == "ExternalInput":
            if name != partition_name:
                in_names.append(name)
        elif alloc.kind == "ExternalOutput":
            out_names.append(name)
            shape = tuple(alloc.tensor_shape)
            dtype = mybir.dt.np(alloc.dtype)
            out_avals.append(jax.core.ShapedArray(shape, dtype))
            zero_shapes.append((shape, dtype))
    n_params = len(in_names)
    n_outs = len(out_avals)
    all_in_names = list(in_names) + list(out_names)
    if partition_name is not None:
        all_in_names.append(partition_name)
    donate = tuple(range(n_params, n_params + n_outs))

    def _body(*args):
        operands = list(args)
        if partition_name is not None:
            operands.append(partition_id_tensor())
        outs = _bass_exec_p.bind(
            *operands,
            out_avals=tuple(out_avals),
            in_names=tuple(all_in_names),
            out_names=tuple(out_names),
            lowering_input_output_aliases=(),
            sim_require_finite=True,
            sim_require_nnan=True,
            nc=nc,
        )
        return tuple(outs)

    devices = jax.devices("axon")[:n_cores]
    mesh = Mesh(np.asarray(devices), ("core",))
    in_specs = (PartitionSpec("core"),) * (n_params + n_outs)
    out_specs = (PartitionSpec("core"),) * len(out_names)
    sharded = jax.jit(
        shard_map(_body, mesh=mesh, in_specs=in_specs, out_specs=out_specs,
                  check_rep=False),
        donate_argnums=donate, keep_unused=True)

    def run(in_maps):
        concat_in = [
            np.concatenate([np.asarray(m[name]) for m in in_maps], axis=0)
            for name in in_names
        ]
        concat_zeros = [np.zeros((n_cores * s[0], *s[1:]), d) for s, d in zero_shapes]
        out_arrs = sharded(*concat_in, *concat_zeros)
        jax.block_until_ready(out_arrs)
        return [
            {name: np.asarray(out_arrs[i]).reshape(n_cores, *out_avals[i].shape)[c]
             for i, name in enumerate(out_names)}
            for c in range(n_cores)
        ]

    return run


def _permute_gates(a):
    # old G index: gate*1024 + half*512 + cg*128 + j  ->  new (half, cg, gate, j)
    v = a.reshape(4, 2, 4, 128, *a.shape[1:])
    v = v.transpose(1, 2, 0, 3, *range(4, v.ndim))
    return np.ascontiguousarray(v.reshape(a.shape))


def _prep_inputs(features, captions, emb_table, W_ih, W_hh, b_ih, b_hh, fc_W, fc_b):
    features = np.asarray(features, dtype=np.float32)
    captions = np.asarray(captions)
    emb_table = np.asarray(emb_table, dtype=np.float32)
    W_ih = _permute_gates(np.asarray(W_ih, dtype=np.float32))
    W_hh = _permute_gates(np.asarray(W_hh, dtype=np.float32))
    b = _permute_gates(
        np.asarray(b_ih, dtype=np.float32) + np.asarray(b_hh, dtype=np.float32))
    fc_W = np.asarray(fc_W, dtype=np.float32)
    fc_b = np.asarray(fc_b, dtype=np.float32)

    embedded = emb_table[captions.astype(np.int64)]          # [B, T, EMB]
    lstm_in = np.concatenate([features, embedded], axis=-1)  # [B, T, DIN]

    def to_sbuf_layout(mat):
        # [K*P, N] -> [P, K*N]: partition-major tiles for contiguous DMA
        kp, n = mat.shape
        return np.ascontiguousarray(
            mat.reshape(kp // P, P, n).transpose(1, 0, 2).reshape(P, -1)
            .astype(ml_dtypes.bfloat16))

    w_ih_T = W_ih.T.astype(np.float32)                       # [DIN, G]
    w_ih_b = np.stack([to_sbuf_layout(w_ih_T[:, n * 512:(n + 1) * 512])
                       for n in range(8)])                   # [8, P, KT*512]
    w_hh_a = to_sbuf_layout(W_hh.T)                          # [P, KT*G]
    bias_rep = np.ascontiguousarray(
        np.broadcast_to(b.astype(ml_dtypes.bfloat16), (P, G)))

    in_maps = []
    for c in range(NCORES):
        xc = lstm_in[c * BL:(c + 1) * BL]                    # [BL, T, DIN]
        x_a = to_sbuf_layout(xc.transpose(2, 1, 0).reshape(DIN, RL))
        fc_w_a = to_sbuf_layout(fc_W[c * VL:(c + 1) * VL].T)
        fcb_rep = np.ascontiguousarray(np.broadcast_to(
            fc_b[c * VL:(c + 1) * VL].astype(ml_dtypes.bfloat16), (P, VL)))
        in_maps.append({
            "x_a": x_a, "w_ih_b": w_ih_b, "w_hh_a": w_hh_a, "bias_rep": bias_rep,
            "fc_w_a": fc_w_a, "fc_b_rep": fcb_rep,
        })
    return in_maps


def _row_perm():
    # device row r = t*256 + (a//4)*128 + (a%4)*32 + b ; bg = a*32 + b
    perm = np.empty(B * T, dtype=np.int64)
    for a in range(NCORES):
        for b in range(BL):
            bg = a * BL + b
            for t in range(T):
                perm[bg * T + t] = t * 256 + (a // 4) * 128 + (a % 4) * 32 + b
    return perm


_PERM = _row_perm()


def _unshard(results):
    out = np.empty((B, T, VOCAB), dtype=np.float32)
    for c in range(NCORES):
        lg = results[c]["logits"][_PERM]                     # [B*T, VL]
        out[:, :, c * VL:(c + 1) * VL] = lg.reshape(B, T, VL)
    return out


def kernel(features, captions, emb_table, W_ih, W_hh, b_ih, b_hh, fc_W, fc_b):
    if "nc" not in _CACHE:
        _CACHE["nc"] = _build_nc()
    if "run" not in _CACHE:
        _CACHE["run"] = _build_sharded(_CACHE["nc"])
    in_maps = _prep_inputs(features, captions, emb_table, W_ih, W_hh, b_ih, b_hh,
                           fc_W, fc_b)
    results = _CACHE["run"](in_maps)
    return _unshard(results)


def kernel_traced(features, captions, emb_table, W_ih, W_hh, b_ih, b_hh, fc_W, fc_b):
    """Same computation via run_bass_kernel_spmd(trace=True); returns
    (output, BassKernelResults) so the caller can read exec_time_ns."""
    from concourse.bass_utils import run_bass_kernel_spmd
    if "nc" not in _CACHE:
        _CACHE["nc"] = _build_nc()
    in_maps = _prep_inputs(features, captions, emb_table, W_ih, W_hh, b_ih, b_hh,
                           fc_W, fc_b)
    res = run_bass_kernel_spmd(_CACHE["nc"], in_maps, list(range(NCORES)), trace=True)
    return _unshard(res.results), res


# revision 28
# speedup vs baseline: 1.0362x; 1.0362x over previous
"""Trainium2 Bass kernel for nn_Decoder (embed -> LSTM -> vocab projection).

v3 layout (8 NeuronCores, single SPMD NEFF):
  - Host: embedding gather + concat -> lstm_in; pre-transpose weights and
    permute the 4096 gate columns into (half, sub, gate, j) order so the
    LSTM gate matmuls can be packed 4x into PE column groups.
  - gx GEMM rows-layout as v2: gx[320,4096] = x @ W_ih^T + b.
  - LSTM recurrence: per half (512 h-cols), ONE psum [128,512] holds 4
    column-group strips; strip cg = h-subchunk cg with free dim (gate,j).
    The 4 strips' matmuls run concurrently on the PE (col-group tiling),
    cutting per-step PE time ~4x vs v2. Activations/elemwise run at full
    128-partition width. One 128x128 PE transpose per half rebuilds h_T.
  - Per-timestep AllGather of h_T; FC m-tiles [128 rows] gather directly
    from the AG buffer (no staging tiles), fc_W shard resident in SBUF.
  - FC vocab-sharded: logits[2560,3750] = hs @ fc_W_shard^T + fc_b,
    interleaved into LSTM steps to keep the PE dense, bulk after step 9.
  - Host: undo row permutation, concat vocab shards.
"""
import ml_dtypes
import numpy as np
import jax
from jax.sharding import Mesh, PartitionSpec
from jax.experimental.shard_map import shard_map

import concourse.bass as bass
import concourse.mybir as mybir
import concourse.tile as tile
from concourse import bacc
from concourse.bass2jax import _bass_exec_p, install_neuronx_cc_hook, partition_id_tensor
from concourse.masks import make_identity

P = 128
NCORES = 8
B, T, FEAT, EMB, HID, VOCAB = 256, 10, 512, 512, 1024, 30000
DIN = FEAT + EMB          # 1024
G = 4 * HID               # 4096
BL = B // NCORES          # 32 batches per core
RL = BL * T               # 320 rows per core (t-major: r = t*BL + b)
RA = 256 * T              # 2560 rows total (t-major: r = t*256 + j*128 + a%4*32 + b)
VL = VOCAB // NCORES      # 3750 vocab per core
KT = DIN // P             # 8 contraction tiles
NVC = (VL + 511) // 512   # 8 vocab chunks (7x512 + 166)
F32 = mybir.dt.float32
BF16 = mybir.dt.bfloat16
Act = mybir.ActivationFunctionType

_CACHE = {}


def _build_nc():
    nc = bacc.Bacc("TRN2", target_bir_lowering=False, debug=False, num_devices=NCORES)
    x_a = nc.dram_tensor("x_a", [P, KT * RL], BF16, kind="ExternalInput").ap()
    w_ih_b = nc.dram_tensor("w_ih_b", [8, P, KT * 512], BF16,
                            kind="ExternalInput").ap()
    w_hh_a = nc.dram_tensor("w_hh_a", [P, KT * G], BF16, kind="ExternalInput").ap()
    bias_rep = nc.dram_tensor("bias_rep", [P, G], BF16, kind="ExternalInput").ap()
    fc_w_a = nc.dram_tensor("fc_w_a", [P, KT * VL], BF16, kind="ExternalInput").ap()
    fc_b_rep = nc.dram_tensor("fc_b_rep", [P, VL], BF16, kind="ExternalInput").ap()
    logits = nc.dram_tensor("logits", [RA, VL], F32, kind="ExternalOutput").ap()

    MT_X = [(0, 128), (128, 128), (256, 64)]  # (row0, rows) m-tiles of 320
    CH = 2048                                 # dma chunk: elements per partition

    with tile.TileContext(nc) as tc:
        with tc.tile_pool(name="dram", bufs=1, space="DRAM") as dram_pool:
            hs_dram = dram_pool.tile([T, HID, BL], BF16)
            ag_outs = [dram_pool.tile([NCORES, HID, BL], BF16,
                                      addr_space="Shared", name=f"ag_{t}")
                       for t in range(T)]
            gx_dram = dram_pool.tile([3, P, G], BF16)
            warm_in = dram_pool.tile([1, 64], BF16)
            warm_out = dram_pool.tile([NCORES, 64], BF16, addr_space="Shared",
                                      name="warm_ag")

            with tc.tile_pool(name="persist", bufs=1) as persist, \
                 tc.tile_pool(name="whh_pool", bufs=1) as whh_pool, \
                 tc.tile_pool(name="fcw_pool", bufs=1) as fcw_pool, \
                 tc.tile_pool(name="hsmt_pool", bufs=3) as hsmt_pool, \
                 tc.tile_pool(name="gates_pool", bufs=2) as gates_pool, \
                 tc.tile_pool(name="step_pool", bufs=2) as step_pool, \
                 tc.tile_pool(name="gxb_pool", bufs=2) as gxb_pool, \
                 tc.tile_pool(name="fc_out", bufs=3) as fc_out, \
                 tc.tile_pool(name="gch_psum", bufs=2, space="PSUM") as gch_psum, \
                 tc.tile_pool(name="tp_psum", bufs=2, space="PSUM") as tp_psum:
                ident_f = persist.tile([P, P], F32)
                make_identity(nc, ident_f[:])
                ident_b = persist.tile([P, P], BF16)
                nc.vector.tensor_copy(ident_b[:], ident_f[:])
                gx_t0 = persist.tile([32, G], BF16)
                h_Tb = persist.tile([P, KT, BL], BF16)
                c_sb = persist.tile([P, 2, P], F32)
                whh_sb = whh_pool.tile([P, KT, G], BF16)
                fcw_sb = fcw_pool.tile([P, KT, VL], BF16)
                fcb_sb = persist.tile([P, VL], BF16)

                whh_flat = whh_sb[:].rearrange("p k g -> p (k g)")
                fcw_flat = fcw_sb[:].rearrange("p k v -> p (k v)")

                # warmup collective: aligns core launch skew off the
                # critical path so AG_0 doesn't pay it after step 0
                nc.gpsimd.dma_start(warm_in[:], x_a[0:1, 0:64])
                nc.gpsimd.collective_compute(
                    "AllGather", mybir.AluOpType.bypass,
                    replica_groups=[list(range(NCORES))],
                    ins=[warm_in.opt()], outs=[warm_out.opt()])

                # ---- FC machinery (vocab-sharded, m-tile = (t, j)) ----
                def load_mtile(t, j):
                    hsmt = hsmt_pool.tile([P, KT, P], BF16,
                                          name=f"hsmt_{t}_{j}", tag="hsmt")
                    for a in range(4):
                        nc.sync.dma_start(
                            hsmt[:, :, 32 * a:32 * a + 32],
                            ag_outs[t][4 * j + a].rearrange("(k p) b -> p k b", p=P))
                    return hsmt

                in_drain = [False]

                def emit_group(hsmt, t, j, v):
                    w = min(512, VL - 512 * v)
                    ps = fc_pools["psum"].tile([P, 512], F32,
                                               name=f"fps_{t}_{j}_{v}", tag="fps")
                    for k in range(KT):
                        nc.tensor.matmul(
                            ps[:, 0:w], hsmt[:, k, :],
                            fcw_sb[:, k, 512 * v:512 * v + w],
                            start=(k == 0), stop=(k == KT - 1))
                    ot = fc_out.tile([P, 512], F32, name=f"fo_{t}_{j}_{v}", tag="fo")
                    nc.vector.tensor_add(ot[:, 0:w], ps[:, 0:w],
                                         fcb_sb[:, 512 * v:512 * v + w])
                    eng = nc.gpsimd if (in_drain[0] and v % 2 == 1) else nc.scalar
                    eng.dma_start(
                        logits[256 * t + P * j:256 * t + P * (j + 1),
                               512 * v:512 * v + w],
                        ot[:, 0:w])

                fc_pools = {}
                fc_state = {"emit": 0, "v": 0, "pref": 0}
                fc_tiles = {}

                def prefetch_fc(t_limit):
                    # issue gather DMAs up to 3 m-tiles ahead (pool WAR paces)
                    while (fc_state["pref"] < 2 * T
                           and fc_state["pref"] - fc_state["emit"] < 3
                           and fc_state["pref"] // 2 <= t_limit):
                        u = fc_state["pref"]
                        fc_tiles[u] = load_mtile(u // 2, u % 2)
                        fc_state["pref"] += 1

                def fc_lim(t):
                    return t - 5 if fc_state["emit"] == 0 else t - 4

                def emit_fc(n_groups, t_limit):
                    done = 0
                    while done < n_groups and fc_state["emit"] < 2 * T:
                        u = fc_state["emit"]
                        t, j = u // 2, u % 2
                        if t > t_limit or u not in fc_tiles:
                            return
                        emit_group(fc_tiles[u], t, j, fc_state["v"])
                        done += 1
                        fc_state["v"] += 1
                        if fc_state["v"] == NVC:
                            fc_state["v"] = 0
                            del fc_tiles[u]
                            fc_state["emit"] += 1

                # ---- Phase A: gx = x @ W_ih^T + b   (rows x gates, bf16) ----
                with tc.tile_pool(name="phA", bufs=1) as phA, \
                     tc.tile_pool(name="wih_pool", bufs=2) as wih_pool, \
                     tc.tile_pool(name="gx_stage", bufs=4) as gx_stage, \
                     tc.tile_pool(name="gx_psum", bufs=2, space="PSUM") as gx_psum:
                    x_sb = phA.tile([P, KT, RL], BF16)
                    x_flat = x_sb[:].rearrange("p k r -> p (k r)")
                    bias_sb = phA.tile([P, G], BF16)
                    # x first (gx starts on it), then wih/whh interleave on
                    # both rings, bias after the first wih pair, fcw + fcb
                    # trickle behind the gx loop.
                    for c0 in range(0, KT * RL, CH):
                        nc.scalar.dma_start(x_flat[:, c0:min(c0 + CH, KT * RL)],
                                            x_a[:, c0:min(c0 + CH, KT * RL)])

                    whh_ci = [0]

                    def trickle_whh(k):
                        for _ in range(k):
                            if whh_ci[0] >= KT * G // CH:
                                return
                            c0 = whh_ci[0] * CH
                            eng = nc.sync if whh_ci[0] % 2 == 0 else nc.scalar
                            eng.dma_start(whh_flat[:, c0:c0 + CH],
                                          w_hh_a[:, c0:c0 + CH])
                            whh_ci[0] += 1

                    def load_wih(n):
                        wt = wih_pool.tile([P, KT, 512], BF16,
                                           name=f"wih_{n}", tag="wih")
                        wf = wt[:].rearrange("p k v -> p (k v)")
                        eng = nc.sync if n % 2 == 0 else nc.scalar
                        for c0 in range(0, KT * 512, CH):
                            eng.dma_start(wf[:, c0:c0 + CH],
                                          w_ih_b[n][:, c0:c0 + CH])
                        return wt

                    nfc = (KT * VL + CH - 1) // CH
                    fcw_ci = [0]

                    def trickle_fcw(k):
                        # ring-paced: these sit behind WAR-gated wih loads /
                        # data-gated evictions, so they enqueue progressively
                        # instead of flooding the hardware queues upfront.
                        for _ in range(k):
                            if fcw_ci[0] >= nfc:
                                return
                            c0 = fcw_ci[0] * CH
                            c1 = min(c0 + CH, KT * VL)
                            eng = nc.sync if fcw_ci[0] % 2 == 0 else nc.scalar
                            eng.dma_start(fcw_flat[:, c0:c1], fc_w_a[:, c0:c1])
                            fcw_ci[0] += 1

                    for c0 in range(0, G, CH):
                        nc.scalar.dma_start(bias_sb[:, c0:c0 + CH],
                                            bias_rep[:, c0:c0 + CH])
                    wts = [load_wih(0), load_wih(1)]
                    trickle_whh(2)
                    for n in range(8):
                        wt = wts[n]
                        if n + 2 < 8:
                            wts.append(load_wih(n + 2))
                        trickle_whh(2)
                        for mi, (r0, rn) in enumerate(MT_X):
                            ps = gx_psum.tile([P, 512], F32,
                                              name=f"gxps_{n}_{mi}", tag="gxps")
                            for k in range(KT):
                                nc.tensor.matmul(
                                    ps[0:rn, :], x_sb[:, k, r0:r0 + rn],
                                    wt[:, k, :],
                                    start=(k == 0), stop=(k == KT - 1))
                            gt = gx_stage.tile([P, 512], BF16,
                                               name=f"gxs_{n}_{mi}", tag="gxs")
                            nc.vector.tensor_add(
                                gt[0:rn, :],
                                ps[0:rn, :], bias_sb[0:rn, n * 512:(n + 1) * 512])
                            if mi == 0:
                                nc.vector.tensor_copy(
                                    gx_t0[:, n * 512:(n + 1) * 512], gt[0:32, :])
                            nc.scalar.dma_start(
                                gx_dram[mi, 0:rn, n * 512:(n + 1) * 512], gt[0:rn, :])
                        trickle_fcw(2)
                        if n == 7:
                            nc.sync.dma_start(fcb_sb[:, 0:CH],
                                              fc_b_rep[:, 0:CH])
                            nc.scalar.dma_start(fcb_sb[:, CH:VL],
                                                fc_b_rep[:, CH:VL])
                    trickle_fcw(nfc)

                # ---- Phase B: LSTM recurrence, FC interleaved ----
                fc_psum_cm = tc.tile_pool(name="fc_psum", bufs=3, space="PSUM")
                fc_pools["psum"] = fc_psum_cm.__enter__()
                for t in range(T):
                    mt, jj = t // 4, t % 4
                    if t > 0:
                        gxt = gxb_pool.tile([32, G], BF16,
                                            name=f"gxt_{t}", tag="gxt")
                        nc.scalar.dma_start(
                            gxt[:], gx_dram[mt, 32 * jj:32 * jj + 32, :])
                        src = gxt
                    else:
                        src = gx_t0

                    gates = {}
                    for half in (0, 1):
                        ps = gch_psum.tile([P, 512], F32,
                                           name=f"gps_{t}_{half}", tag="gps")
                        for cg in range(4):
                            nc.tensor.matmul(
                                ps[32 * cg:32 * cg + 32, :],
                                ident_b[0:32, 0:32],
                                src[:, half * 2048 + cg * 512:
                                    half * 2048 + (cg + 1) * 512],
                                start=True, stop=(t == 0),
                                tile_position=(0, 32 * cg))
                        if t > 0:
                            for k in range(KT):
                                for cg in range(4):
                                    nc.tensor.matmul(
                                        ps[32 * cg:32 * cg + 32, :],
                                        h_Tb[:, k, :],
                                        whh_sb[:, k, half * 2048 + cg * 512:
                                               half * 2048 + (cg + 1) * 512],
                                        start=False, stop=(k == KT - 1),
                                        tile_position=(0, 32 * cg))
                        g_sb = gates_pool.tile([P, 512], F32,
                                               name=f"gates_{t}_{half}", tag="ga")
                        nc.scalar.activation(g_sb[:, 256:384], ps[:, 256:384],
                                             Act.Tanh)
                        nc.scalar.activation(g_sb[:, 0:256], ps[:, 0:256],
                                             Act.Sigmoid)
                        nc.scalar.activation(g_sb[:, 384:512], ps[:, 384:512],
                                             Act.Sigmoid)
                        gates[half] = g_sb

                    emit_fc(1, fc_lim(t))
                    for half in (0, 1):
                        g_sb = gates[half]
                        tmp = step_pool.tile([P, P], F32,
                                             name=f"tmp_{t}_{half}", tag="tmp")
                        nc.vector.tensor_mul(tmp[:], g_sb[:, 0:128],
                                             g_sb[:, 256:384])
                        if t == 0:
                            nc.vector.tensor_copy(c_sb[:, half, :], tmp[:])
                        else:
                            nc.vector.tensor_mul(c_sb[:, half, :],
                                                 g_sb[:, 128:256],
                                                 c_sb[:, half, :])
                            nc.vector.tensor_add(c_sb[:, half, :],
                                                 c_sb[:, half, :], tmp[:])
                        th = step_pool.tile([P, P], F32,
                                            name=f"th_{t}_{half}", tag="th")
                        nc.scalar.activation(th[:], c_sb[:, half, :], Act.Tanh)
                        h_hf = step_pool.tile([P, P], BF16,
                                              name=f"h_{t}_{half}", tag="h")
                        nc.vector.tensor_mul(h_hf[:], g_sb[:, 384:512], th[:])
                        tp = tp_psum.tile([P, P], BF16,
                                          name=f"tp_{t}_{half}", tag="tp")
                        nc.tensor.transpose(tp[:], h_hf[:], ident_b[:])
                        nc.vector.tensor_copy(
                            h_Tb[:, 4 * half:4 * half + 4, :],
                            tp[:].rearrange("p (k b) -> p k b", k=4))
                        nc.scalar.dma_start(
                            hs_dram[t].rearrange(
                                "(k p) b -> p k b", p=P)[:, 4 * half:4 * half + 4, :],
                            h_Tb[:, 4 * half:4 * half + 4, :])
                        if half == 0:
                            emit_fc(1, fc_lim(t))

                    nc.gpsimd.collective_compute(
                        "AllGather", mybir.AluOpType.bypass,
                        replica_groups=[list(range(NCORES))],
                        ins=[hs_dram[t].opt()], outs=[ag_outs[t].opt()])
                    prefetch_fc(t - 2)
                    emit_fc(2, fc_lim(t))

                # ---- Phase C: drain remaining FC work ----
                in_drain[0] = True
                while fc_state["emit"] < 2 * T:
                    prefetch_fc(T - 1)
                    emit_fc(1, T - 1)
                fc_psum_cm.__exit__(None, None, None)
    nc.compile()
    return nc


def _build_sharded(nc, n_cores=NCORES):
    install_neuronx_cc_hook()
    partition_name = nc.partition_id_tensor.name if nc.partition_id_tensor else None
    in_names, out_names, out_avals, zero_shapes = [], [], [], []
    for alloc in nc.m.functions[0].allocations:
        if not isinstance(alloc, mybir.MemoryLocationSet):
            continue
        name = alloc.memorylocations[0].name
        if alloc.kind == "ExternalInput":
            if name != partition_name:
                in_names.append(name)
        elif alloc.kind == "ExternalOutput":
            out_names.append(name)
            shape = tuple(alloc.tensor_shape)
            dtype = mybir.dt.np(alloc.dtype)
            out_avals.append(jax.core.ShapedArray(shape, dtype))
            zero_shapes.append((shape, dtype))
    n_params = len(in_names)
    n_outs = len(out_avals)
    all_in_names = list(in_names) + list(out_names)
    if partition_name is not None:
        all_in_names.append(partition_name)
    donate = tuple(range(n_params, n_params + n_outs))

    def _body(*args):
        operands = list(args)
        if partition_name is not None:
            operands.append(partition_id_tensor())
        outs = _bass_exec_p.bind(
            *operands,
            out_avals=tuple(out_avals),
            in_names=tuple(all_in_names),
            out_names=tuple(out_names),
            lowering_input_output_aliases=(),
            sim_require_finite=True,
            sim_require_nnan=True,
            nc=nc,
        )
        return tuple(outs)

    devices = jax.devices("axon")[:n_cores]
    mesh = Mesh(np.asarray(devices), ("core",))
    in_specs = (PartitionSpec("core"),) * (n_params + n_outs)
    out_specs = (PartitionSpec("core"),) * len(out_names)
    sharded = jax.jit(
        shard_map(_body, mesh=mesh, in_specs=in_specs, out_specs=out_specs,
                  check_rep=False),
        donate_argnums=donate, keep_unused=True)

    def run(in_maps):
        concat_in = [
            np.concatenate([np.asarray(m[name]) for m in in_maps], axis=0)
            for name in in_names
        ]
        concat_zeros = [np.zeros((n_cores * s[0], *s[1:]), d) for s, d in zero_shapes]
        out_arrs = sharded(*concat_in, *concat_zeros)
        jax.block_until_ready(out_arrs)
        return [
            {name: np.asarray(out_arrs[i]).reshape(n_cores, *out_avals[i].shape)[c]
             for i, name in enumerate(out_names)}
            for c in range(n_cores)
        ]

    return run


def _permute_gates(a):
    # old G index: gate*1024 + half*512 + cg*128 + j  ->  new (half, cg, gate, j)
    v = a.reshape(4, 2, 4, 128, *a.shape[1:])
    v = v.transpose(1, 2, 0, 3, *range(4, v.ndim))
    return np.ascontiguousarray(v.reshape(a.shape))


def _prep_inputs(features, captions, emb_table, W_ih, W_hh, b_ih, b_hh, fc_W, fc_b):
    features = np.asarray(features, dtype=np.float32)
    captions = np.asarray(captions)
    emb_table = np.asarray(emb_table, dtype=np.float32)
    W_ih = _permute_gates(np.asarray(W_ih, dtype=np.float32))
    W_hh = _permute_gates(np.asarray(W_hh, dtype=np.float32))
    b = _permute_gates(
        np.asarray(b_ih, dtype=np.float32) + np.asarray(b_hh, dtype=np.float32))
    fc_W = np.asarray(fc_W, dtype=np.float32)
    fc_b = np.asarray(fc_b, dtype=np.float32)

    embedded = emb_table[captions.astype(np.int64)]          # [B, T, EMB]
    lstm_in = np.concatenate([features, embedded], axis=-1)  # [B, T, DIN]

    def to_sbuf_layout(mat):
        # [K*P, N] -> [P, K*N]: partition-major tiles for contiguous DMA
        kp, n = mat.shape
        return np.ascontiguousarray(
            mat.reshape(kp // P, P, n).transpose(1, 0, 2).reshape(P, -1)
            .astype(ml_dtypes.bfloat16))

    w_ih_T = W_ih.T.astype(np.float32)                       # [DIN, G]
    w_ih_b = np.stack([to_sbuf_layout(w_ih_T[:, n * 512:(n + 1) * 512])
                       for n in range(8)])                   # [8, P, KT*512]
    w_hh_a = to_sbuf_layout(W_hh.T)                          # [P, KT*G]
    bias_rep = np.ascontiguousarray(
        np.broadcast_to(b.astype(ml_dtypes.bfloat16), (P, G)))

    in_maps = []
    for c in range(NCORES):
        xc = lstm_in[c * BL:(c + 1) * BL]                    # [BL, T, DIN]
        x_a = to_sbuf_layout(xc.transpose(2, 1, 0).reshape(DIN, RL))
        fc_w_a = to_sbuf_layout(fc_W[c * VL:(c + 1) * VL].T)
        fcb_rep = np.ascontiguousarray(np.broadcast_to(
            fc_b[c * VL:(c + 1) * VL].astype(ml_dtypes.bfloat16), (P, VL)))
        in_maps.append({
            "x_a": x_a, "w_ih_b": w_ih_b, "w_hh_a": w_hh_a, "bias_rep": bias_rep,
            "fc_w_a": fc_w_a, "fc_b_rep": fcb_rep,
        })
    return in_maps


def _row_perm():
    # device row r = t*256 + (a//4)*128 + (a%4)*32 + b ; bg = a*32 + b
    perm = np.empty(B * T, dtype=np.int64)
    for a in range(NCORES):
        for b in range(BL):
            bg = a * BL + b
            for t in range(T):
                perm[bg * T + t] = t * 256 + (a // 4) * 128 + (a % 4) * 32 + b
    return perm


_PERM = _row_perm()


def _unshard(results):
    out = np.empty((B, T, VOCAB), dtype=np.float32)
    for c in range(NCORES):
        lg = results[c]["logits"][_PERM]                     # [B*T, VL]
        out[:, :, c * VL:(c + 1) * VL] = lg.reshape(B, T, VL)
    return out


def kernel(features, captions, emb_table, W_ih, W_hh, b_ih, b_hh, fc_W, fc_b):
    if "nc" not in _CACHE:
        _CACHE["nc"] = _build_nc()
    if "run" not in _CACHE:
        _CACHE["run"] = _build_sharded(_CACHE["nc"])
    in_maps = _prep_inputs(features, captions, emb_table, W_ih, W_hh, b_ih, b_hh,
                           fc_W, fc_b)
    results = _CACHE["run"](in_maps)
    return _unshard(results)


def kernel_traced(features, captions, emb_table, W_ih, W_hh, b_ih, b_hh, fc_W, fc_b):
    """Same computation via run_bass_kernel_spmd(trace=True); returns
    (output, BassKernelResults) so the caller can read exec_time_ns."""
    from concourse.bass_utils import run_bass_kernel_spmd
    if "nc" not in _CACHE:
        _CACHE["nc"] = _build_nc()
    in_maps = _prep_inputs(features, captions, emb_table, W_ih, W_hh, b_ih, b_hh,
                           fc_W, fc_b)
    res = run_bass_kernel_spmd(_CACHE["nc"], in_maps, list(range(NCORES)), trace=True)
    return _unshard(res.results), res
